# revision 2
# baseline (speedup 1.0000x reference)
"""Bass/Trainium2 kernel for DynamicGraphConv (GNN message passing).

Computes, for a graph with N nodes / E edges:
    ns  = segment_sum(x[row], col, N)            # scatter-add of source features
    h   = concat([x, ns], -1) @ W + b
    out = LayerNorm(h) * gamma + beta

Distribution: nodes (and segment targets) are sharded across 8 NeuronCores;
edges are partitioned by destination-node shard so aggregation is local to
each core; the full x is replicated to every core's DRAM for the
source-feature gather (host-side "all-gather").

Per-core pipeline:
  - dma_gather streams x[row] for the core's edges into SBUF (messages) in
    bf16 at 128B per descriptor (half the f32 baseline's gather traffic).
    The gather stride must be a multiple of 256B, so rows are addressed as
    pairs: bucket (r, p) covers sources in range r*50000..(r+1)*50000 with
    parity p, base address = row r*50000+p, stride = 2 rows, int16 index =
    (src - r*50000) >> 1.  elem_size_bytes=128 is below bass's dma_gather
    assert (256B min) but fully supported by the Q7 ucode, so the
    instruction is emitted directly.
  - per 128-destination window, a one-hot matrix U[msg, slot] is built with
    one DVE is_equal op; PE accumulates nsT[64, 128] = sum over msg tiles of
    msgs^T @ U in PSUM (the scatter-add expressed as matmuls).
  - h = [x;1]^T @ [W1;b] + nsT^T @ W2 fused into one PSUM tile per window;
    LayerNorm is applied batched over groups of 8 windows.
"""

import sys

sys.path.insert(0, "/opt/trn_rl_repo")

import numpy as np

# ---- problem constants (hardcoded per contract) ----
N_NODES = 100000
N_EDGES = 1000000
D = 64
OUT = 64
EPS = 1e-5
N_CORES = 8

S = N_NODES // N_CORES          # nodes per core = 12500
WIN = 128                       # dest window size
NWIN = (S + WIN - 1) // WIN     # windows per core = 98
NBUCKET = 4                     # (range r in {0,1}) x (parity p in {0,1})
HALF = N_NODES // 2             # 50000 source rows per range bucket
XG_ROWS = N_NODES + 2           # padded so every bucket's AP covers 25000 pairs
CHUNK_TILES = 16                # msg tiles per gather call (2048 idx)
HBATCH = 8                      # windows per LayerNorm batch


# --------------------------------------------------------------------------
# Host-side preprocessing: shared program structure + per-core input tensors
# --------------------------------------------------------------------------

def host_prep(x, edge_index):
    import ml_dtypes

    x = np.asarray(x, np.float32)
    ei = np.asarray(edge_index)
    row = ei[0].astype(np.int64)
    col = ei[1].astype(np.int64)

    core = col // S
    per_core = []
    for c in range(N_CORES):
        m = core == c
        src = row[m]
        dloc = col[m] - c * S
        win = dloc // WIN
        slot = dloc % WIN
        buck = (src // HALF) * 2 + (src % 2)
        order = np.lexsort((src, win, buck))
        per_core.append((src[order], win[order], slot[order], buck[order]))

    # bucket run lengths, padded to x128 and maxed across cores
    run_len = np.zeros((N_CORES, NBUCKET), np.int64)
    for c in range(N_CORES):
        run_len[c] = np.bincount(per_core[c][3], minlength=NBUCKET)
    run_max = (run_len.max(axis=0) + 127) // 128 * 128
    tiles_per_bucket = run_max // 128
    NT = int(tiles_per_bucket.sum())
    bucket_tile0 = np.concatenate([[0], np.cumsum(tiles_per_bucket)])[:NBUCKET]

    gidx = np.zeros((N_CORES, NT * 128), np.int16)      # within-bucket pair idx
    mwin = np.full((N_CORES, NT * 128), -1, np.int32)   # window of each msg
    mslot = np.full((N_CORES, NT * 128), -1, np.int32)  # slot within window
    lo = np.zeros((N_CORES, NBUCKET, NWIN), np.int64)
    hi = np.zeros((N_CORES, NBUCKET, NWIN), np.int64)
    for c in range(N_CORES):
        srcs, wins, slots, _ = per_core[c]
        starts = np.concatenate([[0], np.cumsum(run_len[c])])
        for b in range(NBUCKET):
            seg = slice(starts[b], starts[b + 1])
            n = int(run_len[c][b])
            base = int(bucket_tile0[b]) * 128
            r = b >> 1
            gidx[c, base:base + n] = ((srcs[seg] - r * HALF) >> 1).astype(np.int16)
            mwin[c, base:base + n] = wins[seg]
            mslot[c, base:base + n] = slots[seg]
            wb = wins[seg]
            lo[c, b] = base + np.searchsorted(wb, np.arange(NWIN))
            hi[c, b] = base + np.searchsorted(wb, np.arange(NWIN), side="right")

    # entries: per window w, per bucket b: union tile span across cores
    entries = []
    entry_windows = []
    win_entry_ofs = [0]
    for w in range(NWIN):
        for b in range(NBUCKET):
            if hi[:, b, w].max() <= lo[:, b, w].min():
                continue
            t0 = int(lo[:, b, w].min()) // 128
            t1 = int((hi[:, b, w].max() + 127) // 128)
            bt0 = int(bucket_tile0[b])
            bt1 = bt0 + int(tiles_per_bucket[b])
            t0, t1 = max(t0, bt0), min(t1, bt1)
            entries.extend(range(t0, t1))
            entry_windows.extend([w] * (t1 - t0))
        win_entry_ofs.append(len(entries))
    entries = np.array(entries, np.int64)
    entry_windows = np.array(entry_windows, np.int64)

    # per-core slot matrix per entry: [128, NENT]
    pos = entries[None, :] * 128 + np.arange(128)[:, None]      # [128, NENT]
    slots_bf = np.empty((N_CORES, 128, len(entries)), ml_dtypes.bfloat16)
    for c in range(N_CORES):
        wmatch = mwin[c][pos] == entry_windows[None, :]
        slots_bf[c] = np.where(wmatch, mslot[c][pos], -1).astype(ml_dtypes.bfloat16)

    # gather calls: chop bucket tile spans into chunks, then interleave the
    # buckets by fractional progress so emission order matches consumption.
    calls = []
    for b in range(NBUCKET):
        t = int(bucket_tile0[b])
        end = t + int(tiles_per_bucket[b])
        while t < end:
            nt = min(CHUNK_TILES, end - t)
            frac = (t - int(bucket_tile0[b])) / max(1, int(tiles_per_bucket[b]))
            calls.append((frac, b, t, nt))
            t += nt
    calls.sort()
    calls = [(b, t, nt) for _, b, t, nt in calls]

    ewmax = int(np.max(np.diff(win_entry_ofs)))
    struct = dict(NT=NT, calls=calls, entries=entries,
                  entry_windows=entry_windows,
                  win_entry_ofs=np.array(win_entry_ofs, np.int64),
                  EWMAX=ewmax)

    iota_bf = np.tile(np.arange(128, dtype=np.float32),
                      (128, 1)).astype(ml_dtypes.bfloat16)
    xg_bf = np.zeros((XG_ROWS, D), ml_dtypes.bfloat16)
    xg_bf[:N_NODES] = x.astype(ml_dtypes.bfloat16)
    per_core_ins = []
    for c in range(N_CORES):
        g = gidx[c]
        gw = np.tile(g.reshape(-1, 16).T, (8, 1)).copy()    # [128, NT*8]
        xt = np.empty((D + 1, S), np.float32)
        xt[:D] = x[c * S:(c + 1) * S].T
        xt[D] = 1.0
        per_core_ins.append(dict(gidx=gw, slots=np.ascontiguousarray(slots_bf[c]),
                                 xt=xt))
    return struct, per_core_ins, dict(iota=iota_bf, xg=xg_bf)


# --------------------------------------------------------------------------
# Bass program
# --------------------------------------------------------------------------

def _dma_gather_128(gp, mybir, out_ap, in_ap, idxs_ap, num_idxs, elem_size,
                    elem_step, queue_num):
    """dma_gather with elem_size_bytes=128 (half the 256B bass minimum).

    Mirrors concourse.bass.BassGpSimd.dma_gather for the DRAM-source,
    non-transpose case.  The Q7 ucode computes descriptor byte counts from
    elem_size directly with no 256B granularity requirement (only the row
    *stride* is encoded in 256B units), so a 128B payload on a 256B stride
    is valid on hardware; bass's `elem_size_bytes % 256 == 0` assert is
    stricter than the ISA.
    """
    gp._assert_queue_num(queue_num)
    assert idxs_ap.dtype == mybir.dt.int16
    assert in_ap.dtype == out_ap.dtype
    stride_bytes = elem_step * mybir.dt.size(in_ap.dtype)
    assert stride_bytes % 256 == 0 and in_ap.ap[0][0] == elem_step
    _in_ap = gp.lower_ap_dma(in_ap, for_custom_bir_dma=True)
    _idxs_ap = gp.lower_ap(idxs_ap)
    _out_ap = gp.lower_ap(out_ap)
    return gp.add_instruction(
        mybir.InstDMAGatherAnt(
            name=gp.bass.get_next_instruction_name(),
            ins=[*_in_ap, _idxs_ap,
                 gp.lower_val_access(gp.to_reg(num_idxs))],
            outs=[_out_ap],
            transpose=False,
            num_idxs=num_idxs,
            elem_size=elem_size,
            stride_bytes_256=stride_bytes // 256,
            gen_mode=0,
            single_packet=False,
            queue_num=queue_num,
        )
    )


def build_program(struct, reps=1, ablate="none"):
    from contextlib import ExitStack
    import concourse.tile as tile
    from concourse import bacc, mybir

    NT = struct["NT"]
    calls = struct["calls"]
    entries = struct["entries"]
    weo = struct["win_entry_ofs"]
    NENT = len(entries)
    EWMAX = struct["EWMAX"]

    nc = bacc.Bacc("TRN2", target_bir_lowering=False, debug=False,
                   num_swdge_queues=4)
    f32, bf16, i16 = mybir.dt.float32, mybir.dt.bfloat16, mybir.dt.int16
    Alu, Act, Ax = mybir.AluOpType, mybir.ActivationFunctionType, mybir.AxisListType

    xg = nc.dram_tensor("xg", [XG_ROWS, D], bf16, kind="ExternalInput")
    gidx = nc.dram_tensor("gidx", [128, NT * 8], i16, kind="ExternalInput")
    slots = nc.dram_tensor("slots", [128, NENT], bf16, kind="ExternalInput")
    xt = nc.dram_tensor("xt", [D + 1, S], f32, kind="ExternalInput")
    w1b = nc.dram_tensor("w1b", [D + 1, OUT], f32, kind="ExternalInput")
    w2 = nc.dram_tensor("w2", [D, OUT], f32, kind="ExternalInput")
    gb = nc.dram_tensor("gb", [128, 2 * OUT], f32, kind="ExternalInput")
    iota = nc.dram_tensor("iota", [128, 128], bf16, kind="ExternalInput")
    out = nc.dram_tensor("out", [S, OUT], f32, kind="ExternalOutput")

    NBATCH = (NWIN + HBATCH - 1) // HBATCH

    tile2call = {}
    for ci, (b, t0, nt) in enumerate(calls):
        for t in range(t0, t0 + nt):
            tile2call[t] = (ci, t - t0)
    win_last_call = []
    running = -1
    for w in range(NWIN):
        ts = entries[weo[w]:weo[w + 1]]
        last = max((tile2call[int(t)][0] for t in ts), default=-1)
        running = max(running, last)
        win_last_call.append(running)

    def bucket_in_ap(b):
        # bucket b=(r,p): pairs of rows starting at row r*HALF+p, stride 2 rows
        base_row = (b >> 1) * HALF + (b & 1)
        return xg.ap()[base_row:base_row + HALF] \
            .rearrange("(n two) d -> n (two d)", two=2)

    with tile.TileContext(nc) as tc, ExitStack() as ctx:
        cpool = ctx.enter_context(tc.tile_pool(name="const", bufs=1))
        mpool = ctx.enter_context(tc.tile_pool(name="msgs", bufs=16))
        upool = ctx.enter_context(tc.tile_pool(name="umat", bufs=3))
        npool = ctx.enter_context(tc.tile_pool(name="nst", bufs=4, space="PSUM"))
        hpool = ctx.enter_context(tc.tile_pool(name="hps", bufs=2, space="PSUM"))
        spool = ctx.enter_context(tc.tile_pool(name="small", bufs=4))
        opool = ctx.enter_context(tc.tile_pool(name="outs", bufs=3))

        gidx_t = cpool.tile([128, NT * 8], i16)
        nc.sync.dma_start(out=gidx_t[:], in_=gidx.ap())
        slots_t = cpool.tile([128, NENT], bf16)
        nc.sync.dma_start(out=slots_t[:], in_=slots.ap())
        xt_t = cpool.tile([D + 1, S], f32)
        nc.sync.dma_start(out=xt_t[:], in_=xt.ap())
        w1b_t = cpool.tile([D + 1, OUT], f32)
        nc.sync.dma_start(out=w1b_t[:], in_=w1b.ap())
        w2_t = cpool.tile([D, OUT], f32)
        nc.sync.dma_start(out=w2_t[:], in_=w2.ap())
        gb_t = cpool.tile([128, 2 * OUT], f32)
        nc.sync.dma_start(out=gb_t[:], in_=gb.ap())
        iota_t = cpool.tile([128, 128], bf16)
        nc.sync.dma_start(out=iota_t[:], in_=iota.ap())
        eps_t = cpool.tile([128, 1], f32)
        nc.vector.memset(eps_t[:], EPS)

        def body():
            chunk_bf = {}
            next_call = [0]
            if ablate == "gather_only":
                for ci in range(len(calls)):
                    b, t0, nt = calls[ci]
                    msgs = mpool.tile([128, CHUNK_TILES, D], bf16, tag="mchunk")
                    _dma_gather_128(
                        nc.gpsimd, mybir,
                        out_ap=msgs[:, :nt, :],
                        in_ap=bucket_in_ap(b),
                        idxs_ap=gidx_t[:, t0 * 8:(t0 + nt) * 8],
                        num_idxs=nt * 128, elem_size=D, elem_step=2 * D,
                        queue_num=ci % 4)
                return

            def emit_call(ci):
                b, t0, nt = calls[ci]
                msgs = mpool.tile([128, CHUNK_TILES, D], bf16, tag="mchunk")
                _dma_gather_128(
                    nc.gpsimd, mybir,
                    out_ap=msgs[:, :nt, :],
                    in_ap=bucket_in_ap(b),
                    idxs_ap=gidx_t[:, t0 * 8:(t0 + nt) * 8],
                    num_idxs=nt * 128, elem_size=D, elem_step=2 * D,
                    queue_num=ci % 4)
                chunk_bf[ci] = msgs

            for batch in range(NBATCH):
                w0 = batch * HBATCH
                wn = min(HBATCH, NWIN - w0)
                hps = hpool.tile([128, HBATCH, OUT], f32)
                for j in range(wn):
                    w = w0 + j
                    tgt = win_last_call[min(w + 3, NWIN - 1)]
                    while next_call[0] <= tgt:
                        emit_call(next_call[0])
                        next_call[0] += 1
                    e0, e1 = int(weo[w]), int(weo[w + 1])
                    ew = e1 - e0
                    nst = npool.tile([OUT, WIN], f32)
                    if ew > 0:
                        U = upool.tile([128, ew, 128], bf16, tag="U")
                        nc.vector.tensor_tensor(
                            out=U[:],
                            in0=slots_t[:, e0:e1]
                                .rearrange("p (e o) -> p e o", o=1)
                                .broadcast_to([128, ew, 128]),
                            in1=iota_t[:].rearrange("p (o k) -> p o k", o=1)
                                .broadcast_to([128, ew, 128]),
                            op=Alu.is_equal)
                        for i, te in enumerate(range(e0, e1)):
                            t = int(entries[te])
                            ci, toff = tile2call[t]
                            nc.tensor.matmul(
                                out=nst[:], lhsT=chunk_bf[ci][:, toff, :],
                                rhs=U[:, i, :],
                                start=(i == 0), stop=(i == ew - 1))
                    else:
                        nc.vector.memset(nst[:], 0.0)
                    nsts = spool.tile([OUT, WIN], f32, tag="nsts")
                    nc.scalar.activation(out=nsts[:], in_=nst[:], func=Act.Copy)
                    nw = min(WIN, S - w * WIN)
                    nc.tensor.matmul(out=hps[:, j, :], lhsT=nsts[:],
                                     rhs=w2_t[:], start=True, stop=False)
                    nc.tensor.matmul(out=hps[:nw, j, :],
                                     lhsT=xt_t[:, w * WIN:w * WIN + nw],
                                     rhs=w1b_t[:], start=False, stop=True)
                # ---- batched LayerNorm over [128, wn, OUT] ----
                red = spool.tile([128, 8, HBATCH], f32, tag="red")
                nmu = red[:, 0, :wn]
                msq = red[:, 1, :wn]
                musq = red[:, 2, :wn]
                var = red[:, 3, :wn]
                std = red[:, 4, :wn]
                rstd = red[:, 5, :wn]
                nmr = red[:, 6, :wn]
                nc.vector.tensor_reduce(out=nmu[:], in_=hps[:, :wn, :],
                                        axis=Ax.X, op=Alu.add, negate=True)
                sq = spool.tile([128, HBATCH, OUT], f32, tag="sq")
                nc.scalar.activation(out=sq[:, :wn, :], in_=hps[:, :wn, :],
                                     func=Act.Square)
                nc.vector.tensor_reduce(out=msq[:], in_=sq[:, :wn, :],
                                        axis=Ax.X, op=Alu.add)
                nc.vector.tensor_scalar(out=nmu[:], in0=nmu[:],
                                        scalar1=1.0 / OUT, scalar2=None,
                                        op0=Alu.mult)
                nc.vector.tensor_scalar(out=msq[:], in0=msq[:],
                                        scalar1=1.0 / OUT, scalar2=None,
                                        op0=Alu.mult)
                nc.vector.tensor_tensor(out=musq[:], in0=nmu[:], in1=nmu[:],
                                        op=Alu.mult)
                nc.vector.tensor_tensor(out=var[:], in0=msq[:], in1=musq[:],
                                        op=Alu.subtract)
                nc.scalar.activation(out=std[:], in_=var[:], func=Act.Sqrt,
                                     bias=eps_t[:])
                nc.vector.reciprocal(out=rstd[:], in_=std[:])
                nc.vector.tensor_tensor(out=nmr[:], in0=nmu[:], in1=rstd[:],
                                        op=Alu.mult)
                z = opool.tile([128, HBATCH, OUT], f32, tag="z")
                for j in range(wn):
                    nc.scalar.activation(out=z[:, j, :], in_=hps[:, j, :],
                                         func=Act.Identity,
                                         bias=nmr[:, j:j + 1],
                                         scale=rstd[:, j:j + 1])
                nc.vector.tensor_tensor(
                    out=z[:, :wn, :], in0=z[:, :wn, :],
                    in1=gb_t[:, :OUT].rearrange("p (o d) -> p o d", o=1)
                        .broadcast_to([128, wn, OUT]),
                    op=Alu.mult)
                nc.vector.tensor_tensor(
                    out=z[:, :wn, :], in0=z[:, :wn, :],
                    in1=gb_t[:, OUT:].rearrange("p (o d) -> p o d", o=1)
                        .broadcast_to([128, wn, OUT]),
                    op=Alu.add)
                nfull = wn if (w0 + wn) * WIN <= S else wn - 1
                if nfull > 0:
                    dst = out.ap()[w0 * WIN:(w0 + nfull) * WIN] \
                        .rearrange("(j p) d -> p j d", p=128)
                    nc.sync.dma_start(out=dst, in_=z[:, :nfull, :])
                if nfull < wn:
                    tail = S - (w0 + nfull) * WIN
                    nc.sync.dma_start(out=out.ap()[(w0 + nfull) * WIN:S],
                                      in_=z[:tail, nfull, :])

        if reps == 1:
            body()
        else:
            with tc.For_i(0, reps, 1,
                          hint_engines=(mybir.EngineType.PE,
                                        mybir.EngineType.DVE,
                                        mybir.EngineType.Pool,
                                        mybir.EngineType.Activation,
                                        mybir.EngineType.SP)):
                body()

    nc.compile()
    return nc


# --------------------------------------------------------------------------
# Entry point
# --------------------------------------------------------------------------

def make_inputs(x, W, b, gamma, beta, struct, per_core, shared):
    w1b_a = np.concatenate([np.asarray(W, np.float32)[:D],
                            np.asarray(b, np.float32)[None, :]], axis=0)
    w2_a = np.ascontiguousarray(np.asarray(W, np.float32)[D:])
    gb_a = np.concatenate([np.tile(np.asarray(gamma, np.float32), (128, 1)),
                           np.tile(np.asarray(beta, np.float32), (128, 1))],
                          axis=1)
    in_maps = []
    for c in range(N_CORES):
        in_maps.append(dict(
            xg=shared["xg"], gidx=per_core[c]["gidx"],
            slots=per_core[c]["slots"], xt=per_core[c]["xt"],
            w1b=w1b_a, w2=w2_a, gb=gb_a, iota=shared["iota"]))
    return in_maps


def kernel(x, edge_index, W, b, gamma, beta):
    from concourse.bass_utils import run_bass_kernel_spmd

    struct, per_core, shared = host_prep(x, edge_index)
    nc = build_program(struct)
    in_maps = make_inputs(x, W, b, gamma, beta, struct, per_core, shared)
    res = run_bass_kernel_spmd(nc, in_maps, core_ids=list(range(N_CORES)))
    out = np.concatenate([res.results[c]["out"] for c in range(N_CORES)],
                         axis=0)
    return out.astype(np.float32)
